# revision 40
# baseline (speedup 1.0000x reference)
"""DeepseekV4 Mega-MoE experts layer on 8 Trainium2 NeuronCores.

Strategy (expert-parallel, per sharding hint):
  - 16 experts sharded 2-per-core across 8 cores; each core receives its two
    experts' weights (losslessly converted: mxfp4*ue8m0 dequant values are
    exactly representable in TRN fp8_e4m3 for both w13 and w2).
  - Staging fp8 quantization of hidden_states runs on the host (direct
    fp32->fp8e4 cast, 1/4 the DMA bytes of fp32); the host also gathers
    tokens per expert (the "all-to-all") and sums per-expert outputs (the
    "combine").

The kernel is HBM-DMA bound (~12.1MB/core at 360 B/ns aggregate ~= 33.7us
of DMA busy vs ~20us PE).  Scheduling (TimelineSim 39.65us vs 42.04us for
the previous version):
  - slot caps sized to the exact max token counts across cores, not 256,
    cutting xgT and ye bytes;
  - one gapless input stream on the SP ring: comb(ACT ring)+xgT, then per
    slot w13 (6 parts) and w2 (3 parts); every ye output transfer is
    emitted after all inputs so input transfers never wait on compute;
  - w13 parts are f-column-major k-pairs (3 gate pairs then 3 up pairs),
    so each pair's PSUM accumulation closes as its part lands and the
    activation chain streams inside the w13 DMA window (no re-read tail);
    mm1 holds only 6 PSUM banks (3 pair-tiles);
  - per-slot compute order mm1(s) -> mm2(s): the non-tail slot's mm2
    finishes mid-stream, so only the tail slot's mm2 p2-pass + copies +
    DMA-issue latency (~2us) sit after the last input byte;
  - mm2 part-major with all 8 banks open as 4 pair-tiles; 1024-wide
    PSUM->SBUF pair copies on ACT/DVE (fused per-token comb*2^9 scale,
    GPSIMD cannot touch PSUM); the tail chunk's ye goes out as two half-D
    transfers on the ACT/Pool rings (issue latencies overlap) and the last
    small chunk rides the idle SP ring.

Per-core device pipeline:
  mm1 hT[f,tok] = w13T chunks x xgT (fp8 DoubleRow, accum over d), one
    k-pair per w13 part; Silu (ACT) right after each gate pair closes.
  a^T = Silu(hT_gate) * hT_up * 2^-9, split hi+lo into TWO fp8 tensors
    (deq(hi)+deq(lo) carries ~8 mantissa bits) so mm2 runs fp8 DoubleRow.
    stt on DVE; hi/lo split on Pool for pairs 0,1 and on DVE for the
    critical pair 2.  Per-pair aT/sil/as2 tiles keep dependencies exact.
  mm2 ye[tok,d] = aT_hi/aT_lo x w2T, part-major (p0 pass over all 8
    groups, then p1, then p2) so the PE tail after the last w2 part is one
    pass; mm2 Ldweights always load [128, 2(stride 256), 128] — the
    dual-fp8 ISA check rejects partial-row loads, so aT is padded to full
    128-token chunks (pad columns are garbage, never copied out).
"""

import os
import sys

if "/opt/trn_rl_repo" not in sys.path:
    sys.path.insert(0, "/opt/trn_rl_repo")

# recover cleanly if a previous process left the NeuronCores wedged
os.environ.setdefault("NEURON_RT_RESET_CORES", "1")

import numpy as np
import ml_dtypes

T, D, I, E, TOPK, GROUP = 512, 2048, 768, 16, 8, 32
N_CORES = 8
E_LOC = E // N_CORES  # experts per core
S_A = 2.0 ** -9       # fixed pre-scale for fp8 hi/lo split of activations

FP8 = ml_dtypes.float8_e4m3      # TRN FP8_EXP4 (max 240) == bass dt.float8e4
BF16 = ml_dtypes.bfloat16

_FP4_TABLE = np.array(
    [0.0, 0.5, 1.0, 1.5, 2.0, 3.0, 4.0, 6.0,
     -0.0, -0.5, -1.0, -1.5, -2.0, -3.0, -4.0, -6.0], dtype=np.float32)


def _dequant_mxfp4(w_packed, sf):
    lo = _FP4_TABLE[w_packed & 0xF]
    hi = _FP4_TABLE[(w_packed >> 4) & 0xF]
    w = np.stack([lo, hi], axis=-1).reshape(*w_packed.shape[:-1], -1)
    s = (sf.astype(np.uint32) << 23).view(np.float32)
    w = w.reshape(*sf.shape, GROUP) * s[..., None]
    return w.reshape(*w_packed.shape[:-1], 2 * w_packed.shape[-1])


_PROGRAM_CACHE = {}

# Tail-slot output routing (picked by TimelineSim sweep): per half-D
# transfer (m0a, m0b, m1a, m1b) the issuing queue, and per chunk whether to
# merge its two halves into one full-D transfer.
_TAIL_QUEUES = ("sp", "pool", "sp", "sp")
_TAIL_MERGE = (False, True)
_TAIL_FLIP = True   # True: DVE takes dqp0 pair copies on the tail slot


def _build_program(caps, split_waits=True):
    """caps: tuple of per-slot token capacities, each <= 256."""
    if isinstance(caps, int):
        caps = (caps,)
    import concourse.bass as bass
    import concourse.mybir as mybir
    import concourse.tile as tile

    _TC = tile.TileContext

    def _split_excess_waits(nc):
        # This walrus build accepts only ONE sem-wait per instruction; hoist
        # extra waits onto standalone EventSemaphore (pure-wait) instructions
        # on the same engine, which execute in order ahead of the original.
        n = 0
        for f in nc.m.functions:
            for b in f.blocks:
                out = []
                for ins in b.instructions:
                    si = ins.sync_info
                    waits = list(si.on_wait) if (si and si.on_wait) else []
                    if len(waits) > 1:
                        for k, w in enumerate(waits[:-1]):
                            out.append(mybir.InstEventSemaphore(
                                name=f"{ins.name}-xw{k}", engine=ins.engine,
                                ins=[], outs=[],
                                sync_info=mybir.SyncInfo(
                                    on_wait=[w], on_update=[])))
                            n += 1
                        si.on_wait = waits[-1:]
                    out.append(ins)
                b.instructions = out
        return n

    dt = mybir.dt
    S = len(caps)
    assert all(c <= 256 for c in caps)
    DT, IT = D // 128, I // 128      # 16, 6
    SLOTS = sum(caps)
    OFF = [sum(caps[:s]) for s in range(S)]          # xgT slot offsets
    MTs = [-(-c // 128) for c in caps]               # chunks per slot
    TOTM = sum(MTs)
    W13P, W2P = 6, 3                 # DMA parts per slot weight
    FH, KH = 2 * I // W13P, IT // W2P  # 256 f-cols (one k-pair), 2 i-tiles
    AF = mybir.ActivationFunctionType

    nc = bass.Bass()
    xgt_d = nc.dram_tensor("xgt", [128, DT, SLOTS], dt.float8e4, kind="ExternalInput")
    # w13 parts are f-column-major (one k-pair per part: 3 gate pairs then 3
    # up pairs) so each pair's PSUM accumulation closes as its part lands.
    w13_d = nc.dram_tensor("w13t", [S, W13P, 128, DT, FH], dt.float8e4, kind="ExternalInput")
    w2_d = nc.dram_tensor("w2t", [S, IT, 128, D], dt.float8e4, kind="ExternalInput")
    comb_d = nc.dram_tensor("combg", [128, TOTM], dt.float32, kind="ExternalInput")
    ye_d = nc.dram_tensor("ye", [TOTM, 128, D], dt.bfloat16, kind="ExternalOutput")

    with _TC(nc) as tc:
        with (
            tc.tile_pool(name="inp", bufs=1) as inp,
            tc.tile_pool(name="wts", bufs=1) as wtsp,
            tc.tile_pool(name="xg", bufs=1) as xgp,
            tc.tile_pool(name="act", bufs=1) as actp,
            tc.tile_pool(name="at", bufs=1) as atp,
            tc.tile_pool(name="yout", bufs=1) as youtp,
            tc.tile_pool(name="ps_h", bufs=1, space="PSUM") as psh,
        ):
            # ---- DMAs in consumption order on the SP ring ----
            # comb rides the ACT queue so it never delays the SP stream.
            combg = inp.tile([128, TOTM], dt.float32, tag="cg")
            nc.scalar.dma_start(combg[:], comb_d[:])
            # xgT: host pre-gathers/transposes (it computes the routing
            # anyway), partition-major so the DMA is one big burst.
            xgT = xgp.tile([128, DT, SLOTS], dt.float8e4, tag="xgT")
            nc.sync.dma_start(xgT[:], xgt_d[:])
            # weights interleaved per slot (w13 s, w2 s) so each slot's mm2
            # runs as early as possible and only the last slot's mm2 + copies
            # sit in the tail behind the final w2 transfer.
            w13t = [[None] * W13P for _ in range(S)]
            w2t = [[None] * W2P for _ in range(S)]
            for s in range(S):
                for p in range(W13P):
                    wt = wtsp.tile([128, DT, FH], dt.float8e4, tag=f"w13_{s}_{p}")
                    nc.sync.dma_start(wt[:], w13_d[s, p])
                    w13t[s][p] = wt
                for p in range(W2P):
                    w2 = wtsp.tile([128, KH, D], dt.float8e4, tag=f"w2_{s}_{p}")
                    nc.sync.dma_start(
                        w2[:], w2_d[s, p * KH:(p + 1) * KH].rearrange("k p f -> p k f"))
                    w2t[s][p] = w2



            def mm1_pair(s, dst_pair, part):
                # One k-pair accumulation: contract all DT d-tiles of w13
                # part `part` against this slot's gathered tokens.  The two
                # k columns go to dst_pair kk=0,1 (separate PSUM banks).
                c = caps[s]
                for u in range(DT // 2):
                    for kk in range(2):
                        nc.tensor.matmul(
                            dst_pair[:, kk, 0:c],
                            w13t[s][part][:, 2 * u:2 * u + 2,
                                          kk * 128:(kk + 1) * 128],
                            xgT[:, 2 * u:2 * u + 2, OFF[s]:OFF[s] + caps[s]],
                            start=(u == 0), stop=(u == DT // 2 - 1),
                            perf_mode=mybir.MatmulPerfMode.DoubleRow)

            # Per slot: mm1 + activation chain streaming the w13 DMA (gate
            # pair v closes as part v lands -> Silu; up pair v closes as part
            # 3+v lands -> stt/cast/sub), then mm2 part-major with all 8
            # PSUM banks open so after the last w2 part lands only that
            # part's pass + copies remain; ye DMAs go last on the SP ring.
            yes = [youtp.tile([128, MTs[s], D], dt.bfloat16, tag=f"ye_{s}",
                              name=f"ye_{s}") for s in range(S)]
            mi = 0
            for s in range(S):
                c = caps[s]
                gps = [psh.tile([128, 2, 512], dt.float32, tag=f"hp{v}",
                                name=f"g{v}_{s}") for v in range(3)]
                # per-pair chain scratch so pairs don't false-serialize on
                # whole-tile dependencies
                sils = [actp.tile([128, 2, c], dt.float32, tag=f"sil{v}",
                                  name=f"sil{v}_{s}") for v in range(3)]
                as2s = [actp.tile([128, 2, c], dt.float32, tag=f"as2{v}",
                                  name=f"as2{v}_{s}") for v in range(3)]
                for v in range(3):
                    mm1_pair(s, gps[v], v)
                    nc.scalar.activation(sils[v][:], gps[v][:, :, 0:c],
                                         AF.Silu)
                # Per-pair aT tiles so mm2's part-p pass depends only on
                # chain pair p.  Width padded to full 128-token chunks: the
                # dual-fp8 Ldweights ISA check rejects partial-row loads, so
                # mm2 always loads [128, 2(stride 256), 128] like mm1.
                cpad = MTs[s] * 128
                aThi = [atp.tile([128, 2, cpad], dt.float8e4, tag=f"aThi_{s}_{v}",
                                 name=f"aThi_{s}_{v}") for v in range(3)]
                aTlo = [atp.tile([128, 2, cpad], dt.float8e4, tag=f"aTlo_{s}_{v}",
                                 name=f"aTlo_{s}_{v}") for v in range(3)]
                for v in range(3):
                    ups = psh.tile([128, 2, 512], dt.float32, tag=f"hp{v}",
                                   name=f"u{v}_{s}")
                    mm1_pair(s, ups, 3 + v)
                    # stt's stay on DVE so they stream with the parts; the
                    # hi/lo split runs on Pool for pairs 0,1 and on DVE for
                    # the critical last pair (no cross-engine hop before mm2)
                    nc.vector.scalar_tensor_tensor(
                        as2s[v][:], sils[v][:], S_A, ups[:, :, 0:c],
                        op0=mybir.AluOpType.mult, op1=mybir.AluOpType.mult)
                    eng = nc.vector if v == 2 else nc.gpsimd
                    eng.tensor_copy(aThi[v][:, :, 0:c], as2s[v][:])
                    eng.tensor_tensor(
                        aTlo[v][:, :, 0:c], as2s[v][:], aThi[v][:, :, 0:c],
                        op=mybir.AluOpType.subtract)
                # all 8 PSUM banks as 4 pair-tiles: group (m, dq) lives in
                # pair 2*m + dq//2, bank dq%2 — so the PSUM->SBUF copies can
                # be 1024-wide pair reads (half the copies and sem hops).
                # Tokens beyond c are garbage fp8; their PSUM rows are never
                # copied out.
                yhp = [psh.tile([128, 2, 512], dt.float32, tag=f"hp{p}",
                                name=f"yhp{p}_{s}") for p in range(4)]
                for p in range(W2P):
                    for m in range(MTs[s]):
                        for dq in range(4):
                            yh = yhp[2 * m + dq // 2][:, dq % 2, :]
                            for at in (aThi[p], aTlo[p]):
                                nc.tensor.matmul(
                                    yh, at[:, :, m * 128:(m + 1) * 128],
                                    w2t[s][p][:, 0:2, dq * 512:(dq + 1) * 512],
                                    start=(p == 0 and at is aThi[p]),
                                    stop=(p == W2P - 1 and at is aTlo[p]),
                                    perf_mode=mybir.MatmulPerfMode.DoubleRow)
                # 1024-wide pair copies on ACT/DVE (GPSIMD cannot access
                # PSUM), m0-first so each ye chunk DMA fires earliest.  On
                # the tail slot DVE (slower) takes the earlier-stopping dqp0
                # pairs so both engines stream without waiting.
                for pi in range(2 * MTs[s]):
                    m, dqp = pi // 2, pi % 2
                    rows = min(128, caps[s] - 128 * m)
                    cg = combg[0:rows, mi + m:mi + m + 1]
                    dst = yes[s][0:rows, m, dqp * 1024:(dqp + 1) * 1024]
                    src = yhp[2 * m + dqp][0:rows]
                    on_act = (pi % 2 == (1 if _TAIL_FLIP else 0)) \
                        if s == S - 1 else (pi % 2 == 0)
                    if on_act:
                        nc.scalar.activation(dst, src, AF.Copy, scale=cg)
                    else:
                        nc.vector.tensor_scalar(dst, src, cg, None,
                                                op0=mybir.AluOpType.mult)
                if s == S - 1:
                    # Tail slot: half-D transfers wait only their own pair
                    # copy, spread across queues so the issue latencies
                    # overlap (config picked by sim sweep)
                    tq = [{"sp": nc.sync, "act": nc.scalar,
                           "pool": nc.gpsimd}[q] for q in _TAIL_QUEUES]
                    for m in range(MTs[s]):
                        rows = min(128, caps[s] - 128 * m)
                        if _TAIL_MERGE[m % len(_TAIL_MERGE)]:
                            tq[2 * m + 1].dma_start(ye_d[mi + m, 0:rows, :],
                                                    yes[s][0:rows, m, :])
                        else:
                            for dqp in range(2):
                                tq[2 * m + dqp].dma_start(
                                    ye_d[mi + m, 0:rows,
                                         dqp * 1024:(dqp + 1) * 1024],
                                    yes[s][0:rows, m,
                                           dqp * 1024:(dqp + 1) * 1024])
                else:
                    for m in range(MTs[s]):
                        rows = min(128, caps[s] - 128 * m)
                        nc.sync.dma_start(ye_d[mi + m, 0:rows, :],
                                          yes[s][0:rows, m, :])
                mi += MTs[s]

    nc.finalize()
    if split_waits:
        _split_excess_waits(nc)
    return nc


def kernel(hidden_states, topk_weights, topk_ids, w13_weight, w13_weight_scale,
           w2_weight, w2_weight_scale):
    from concourse.bass_utils import run_bass_kernel_spmd

    x = np.asarray(hidden_states, dtype=np.float32)
    tw = np.asarray(topk_weights, dtype=np.float32)
    ti = np.asarray(topk_ids)

    # host routing: combine weights + per-expert token lists
    comb = np.zeros((T, E), np.float32)
    for k in range(TOPK):
        np.add.at(comb, (np.arange(T), ti[:, k]), tw[:, k])
    routed = comb > 0.0
    idx = [np.nonzero(routed[:, e])[0] for e in range(E)]

    # per-core slots: each slot is (expert, token subrange) with <= 256
    # tokens; slot 0 gets the larger share so caps stay tight.
    core_slots = []
    for core in range(N_CORES):
        slots = []
        for le in range(E_LOC):
            e = core * E_LOC + le
            n = len(idx[e])
            for st in range(0, max(n, 1), 256):
                slots.append((e, st, min(256, n - st)))
        slots.sort(key=lambda t: -t[2])
        core_slots.append(slots)
    NS = max(len(s) for s in core_slots)
    for slots in core_slots:
        while len(slots) < NS:
            slots.append((0, 0, 0))
    caps = tuple(
        max(16, max(core_slots[c][s][2] for c in range(N_CORES)))
        for s in range(NS))

    if caps not in _PROGRAM_CACHE:
        _PROGRAM_CACHE[caps] = _build_program(caps)
    nc = _PROGRAM_CACHE[caps]

    SLOTS = sum(caps)
    OFF = [sum(caps[:s]) for s in range(NS)]
    MTs = [-(-c // 128) for c in caps]
    TOTM = sum(MTs)
    MOFF = [sum(MTs[:s]) for s in range(NS)]
    DT, IT = D // 128, I // 128

    # host staging quantization + gather/transpose + lossless weight conv
    xq8 = x.astype(FP8)
    w13 = _dequant_mxfp4(np.asarray(w13_weight), np.asarray(w13_weight_scale))
    w2 = _dequant_mxfp4(np.asarray(w2_weight), np.asarray(w2_weight_scale))
    # w13 parts are f-column-major k-pairs: part j = cols [256j, 256j+256)
    # of w13[e].T, laid out [128, DT, 256] partition-major
    W13P, FH = 6, 256
    w13t8 = [np.ascontiguousarray(
        w13[e].T.astype(FP8).reshape(DT, 128, W13P, FH).transpose(2, 1, 0, 3))
        for e in range(E)]
    w2t8 = [w2[e].T.astype(FP8).reshape(IT, 128, D) for e in range(E)]

    in_maps = []
    for core in range(N_CORES):
        m = {}
        xgt = np.zeros((128, DT, SLOTS), FP8)
        cg = np.zeros((128, TOTM), np.float32)
        w13m = np.zeros((NS, W13P, 128, DT, FH), FP8)
        w2m = np.zeros((NS, IT, 128, D), FP8)
        for s, (e, st, n) in enumerate(core_slots[core]):
            if n == 0:
                continue
            ix = idx[e][st:st + n]
            xgt[:, :, OFF[s]:OFF[s] + n] = np.transpose(
                xq8[ix].T.reshape(DT, 128, n), (1, 0, 2))
            cw = comb[ix, e] / S_A      # undo the fp8 pre-scale
            for mm in range(MTs[s]):
                r = min(128, n - 128 * mm)
                if r > 0:
                    cg[0:r, MOFF[s] + mm] = cw[128 * mm:128 * mm + r]
            w13m[s] = w13t8[e]
            w2m[s] = w2t8[e]
        m["xgt"] = xgt
        m["combg"] = cg
        m["w13t"] = w13m
        m["w2t"] = w2m
        in_maps.append(m)

    res = run_bass_kernel_spmd(nc, in_maps, list(range(N_CORES)))

    out = np.zeros((T, D), np.float32)
    for core in range(N_CORES):
        ye = np.asarray(res.results[core]["ye"], dtype=np.float32)
        for s, (e, st, n) in enumerate(core_slots[core]):
            if n == 0:
                continue
            ix = idx[e][st:st + n]
            for mm in range(MTs[s]):
                r = min(128, n - 128 * mm)
                if r > 0:
                    out[ix[128 * mm:128 * mm + r]] += ye[MOFF[s] + mm, 0:r]
    return out
